# revision 12
# baseline (speedup 1.0000x reference)
"""Trainium2 Bass kernel for nn_CascadeClassifier (embedding_lookup).

Computes, for a pair batch B=16384:
  s_score = relu(inp_sf @ W_obj1 + b_obj1) @ W_obj2          [B, 1001]
  o_score = relu(inp_of @ W_obj1 + b_obj1) @ W_obj2          [B, 1001]
  pvf_emb = relu(inp_pvf @ W_pvf + b_pvf)                    [B, 512]
  pf_emb  = relu(concat(pvf_emb, inp_ppf) @ W_pf + b_pf)     [B, 512]
  p_out   = pf_emb @ W_pred + so2p[gt_s*1001+gt_o] * exp(f)  [B, 132]

Sharding: data-parallel over B across 8 NeuronCores (2048 rows each);
MLP weights and the 529 MB so2p table replicated per core.  Activations
and weights are pre-blocked host-side into the exact SBUF tile layout
(feature-on-partition, k-tiles contiguous) so every SWDGE load is a
single max-width descriptor stream and no on-device transposes exist;
hidden activations are produced feature-on-partition (hT layout), which
makes layer biases per-partition ACT bias operands and feeds the next
matmul's stationary side directly.  Matmuls run in fp32r (full PE rate
at moving dim >= 256, ~2e-4 rel err vs 2.5e-3 for bf16).  The so2p
lookup is an on-device indirect-DMA row gather with indices computed on
VectorE.
"""

import numpy as np

# ---- problem constants (hardcoded per contract) ----
B = 16384
NCORES = 8
BC = B // NCORES          # 2048 rows per core
F = 1024                  # OBJ_FEAT / PVF_DIM
H = 512                   # HIDDEN
OBJ = 1001
OBJP = OBJ + 1            # even-padded: fp32r matmul dst needs even free dim
PRED = 132
PPF = 64
CAT = H + PPF             # 576
CHUNK = 512               # batch tile in the matmul free dim
NCHUNK = BC // CHUNK      # 4
SUB = 128                 # batch sub-tile (output partition dim)
NSUB = CHUNK // SUB       # 4
NT = BC // SUB            # 16 index tiles of 128
KF = F // 128             # 8 k-tiles for the F-dim matmuls
KH = H // 128             # 4 k-tiles for the H-dim matmuls
XW = KF * CHUNK           # per-chunk x-tile free width (4096)

_BUILT = None
LAST_RESULT = None


def _build():
    import concourse.bass as bass
    import concourse.tile as tile
    from concourse import bacc, mybir

    f32 = mybir.dt.float32
    f32r = mybir.dt.float32r
    i32 = mybir.dt.int32
    AF = mybir.ActivationFunctionType
    ALU = mybir.AluOpType

    nc = bacc.Bacc("TRN2", target_bir_lowering=False, debug=False,
                   num_devices=NCORES)

    # ---- per-core device IO (host pre-blocked layouts) ----
    sfb = nc.dram_tensor("sfb", [128, NCHUNK * XW], f32r, kind="ExternalInput")
    ofb = nc.dram_tensor("ofb", [128, NCHUNK * XW], f32r, kind="ExternalInput")
    pvfb = nc.dram_tensor("pvfb", [128, NCHUNK * XW], f32r, kind="ExternalInput")
    ppfT = nc.dram_tensor("ppfT", [PPF, BC], f32r, kind="ExternalInput")
    gts = nc.dram_tensor("gts", [SUB, NT], i32, kind="ExternalInput")
    gto = nc.dram_tensor("gto", [SUB, NT], i32, kind="ExternalInput")
    w1 = nc.dram_tensor("w1", [128, KF * H], f32r, kind="ExternalInput")
    b1 = nc.dram_tensor("b1", [128, KH], f32, kind="ExternalInput")
    w2 = nc.dram_tensor("w2", [128, KH * OBJP], f32r, kind="ExternalInput")
    wv = nc.dram_tensor("wv", [128, KF * H], f32r, kind="ExternalInput")
    bv = nc.dram_tensor("bv", [128, KH], f32, kind="ExternalInput")
    wpf = nc.dram_tensor("wpf", [128, KH * H], f32r, kind="ExternalInput")
    wpfb = nc.dram_tensor("wpfb", [PPF, H], f32r, kind="ExternalInput")
    bpf = nc.dram_tensor("bpf", [128, KH], f32, kind="ExternalInput")
    wpred = nc.dram_tensor("wpred", [128, KH * PRED], f32r, kind="ExternalInput")
    so2p = nc.dram_tensor("so2p", [OBJ * OBJ, PRED], f32, kind="ExternalInput")
    factor = nc.dram_tensor("factor", [128, 1], f32, kind="ExternalInput")

    s_score = nc.dram_tensor("s_score", [BC, OBJ], f32, kind="ExternalOutput")
    o_score = nc.dram_tensor("o_score", [BC, OBJ], f32, kind="ExternalOutput")
    p_out = nc.dram_tensor("p_out", [BC, PRED], f32, kind="ExternalOutput")

    with tile.TileContext(nc) as tc:
        with tc.tile_pool(name="wpool", bufs=1) as wpool, \
             tc.tile_pool(name="xpool", bufs=4) as xpool, \
             tc.tile_pool(name="hpool", bufs=18) as hpool, \
             tc.tile_pool(name="opool", bufs=4) as opool, \
             tc.tile_pool(name="spool", bufs=4) as spool, \
             tc.tile_pool(name="pm1", bufs=2, space="PSUM") as pm1, \
             tc.tile_pool(name="pm2", bufs=2, space="PSUM") as pm2, \
             tc.tile_pool(name="pmp", bufs=2, space="PSUM") as pmp:

            # ---- persistent weights ----
            # All fp32r operands are pre-rounded host-side (_round_f32r), so
            # these are plain cast-free DMAs.  Load order = SWDGE queue
            # order; sequenced so the first chunk's matmul dependencies
            # (w1, x_s0, x_o0) land earliest.
            w1_sb = wpool.tile([128, KF * H], f32r)
            nc.gpsimd.dma_start(out=w1_sb[:], in_=w1[:])

            x_sb = {}
            x_eng = {"s": nc.gpsimd, "o": nc.gpsimd, "v": nc.gpsimd}

            def load_x(path, src, c):
                x_t = xpool.tile([128, XW], f32r, tag="x", name=f"x_{path}{c}")
                x_eng[path].dma_start(out=x_t[:], in_=src[:, c * XW:(c + 1) * XW])
                x_sb[(path, c)] = x_t

            load_x("s", sfb, 0)
            load_x("o", ofb, 0)

            w2_sb = wpool.tile([128, KH * OBJP], f32r)
            nc.gpsimd.dma_start(out=w2_sb[:], in_=w2[:])

            load_x("v", pvfb, 0)

            wv_sb = wpool.tile([128, KF * H], f32r)
            nc.gpsimd.dma_start(out=wv_sb[:], in_=wv[:])
            wpf_a = wpool.tile([128, KH * H], f32r)
            nc.gpsimd.dma_start(out=wpf_a[:], in_=wpf[:])
            wpf_b = wpool.tile([PPF, H], f32r)
            nc.gpsimd.dma_start(out=wpf_b[:], in_=wpfb[:])
            wpred_sb = wpool.tile([128, KH * PRED], f32r)
            nc.gpsimd.dma_start(out=wpred_sb[:], in_=wpred[:])
            ppf_sb = wpool.tile([PPF, BC], f32r)
            nc.gpsimd.dma_start(out=ppf_sb[:], in_=ppfT[:])

            # ---- small constants (HWDGE, parallel to the SWDGE queue) ----
            b1_sb = wpool.tile([128, KH], f32)
            nc.sync.dma_start(out=b1_sb[:], in_=b1[:])
            bv_sb = wpool.tile([128, KH], f32)
            nc.sync.dma_start(out=bv_sb[:], in_=bv[:])
            bpf_sb = wpool.tile([128, KH], f32)
            nc.sync.dma_start(out=bpf_sb[:], in_=bpf[:])
            fac_sb = wpool.tile([128, 1], f32)
            nc.sync.dma_start(out=fac_sb[:], in_=factor[:])
            exp_sb = wpool.tile([128, 1], f32)
            nc.scalar.activation(out=exp_sb[:], in_=fac_sb[:], func=AF.Exp)

            # ---- gather indices: idx = gt_s * 1001 + gt_o (int32, exact) ----
            gts_sb = wpool.tile([SUB, NT], i32)
            nc.sync.dma_start(out=gts_sb[:], in_=gts[:])
            gto_sb = wpool.tile([SUB, NT], i32)
            nc.sync.dma_start(out=gto_sb[:], in_=gto[:])
            idx_sb = wpool.tile([SUB, NT], i32)
            nc.vector.tensor_scalar(out=idx_sb[:], in0=gts_sb[:], scalar1=OBJ,
                                    scalar2=None, op0=ALU.mult)
            nc.vector.tensor_tensor(out=idx_sb[:], in0=idx_sb[:], in1=gto_sb[:],
                                    op=ALU.add)

            srcs = {"s": sfb, "o": ofb, "v": pvfb}
            w1s = {"s": w1_sb, "o": w1_sb, "v": wv_sb}
            b1s = {"s": b1_sb, "o": b1_sb, "v": bv_sb}

            def mm1(path, c):
                """layer 1: hT[m] = relu(W.T @ xT + b), 4 tiles [128h, CHUNK]."""
                x_t = x_sb.pop((path, c))
                out = []
                for m in range(KH):
                    ps = pm1.tile([128, CHUNK], f32, tag="pm1",
                                  name=f"ps1_{path}{c}_{m}")
                    for kk in range(KF):
                        nc.tensor.matmul(
                            out=ps[:],
                            lhsT=w1s[path][:, kk * H + m * 128: kk * H + (m + 1) * 128],
                            rhs=x_t[:, kk * CHUNK:(kk + 1) * CHUNK],
                            start=(kk == 0), stop=(kk == KF - 1))
                    h_t = hpool.tile([128, CHUNK], f32r, tag="hT",
                                     name=f"h_{path}{c}_{m}")
                    nc.scalar.activation(out=h_t[:], in_=ps[:], func=AF.Relu,
                                         bias=b1s[path][:, m:m + 1])
                    out.append(h_t)
                return out

            def mm2(path, c, hT):
                """layer 2 for s/o: [128b, OBJ] score tiles -> DRAM."""
                out_dram = s_score if path == "s" else o_score
                for i in range(NSUB):
                    ps2 = pm2.tile([128, OBJP], f32, tag="pm2",
                                   name=f"ps2_{path}{c}_{i}")
                    for kk in range(KH):
                        lhsT = hT[kk][:, i * SUB:(i + 1) * SUB]
                        nc.tensor.matmul(out=ps2[:, 0:512],
                                         lhsT=lhsT,
                                         rhs=w2_sb[:, kk * OBJP: kk * OBJP + 512],
                                         start=(kk == 0), stop=(kk == KH - 1))
                        nc.tensor.matmul(out=ps2[:, 512:OBJP],
                                         lhsT=lhsT,
                                         rhs=w2_sb[:, kk * OBJP + 512:(kk + 1) * OBJP],
                                         start=(kk == 0), stop=(kk == KH - 1))
                    o_t = opool.tile([128, OBJ], f32, tag="out",
                                     name=f"o_{path}{c}_{i}")
                    nc.vector.tensor_copy(out=o_t[:], in_=ps2[:, 0:OBJ])
                    r0 = c * CHUNK + i * SUB
                    nc.sync.dma_start(out=out_dram[r0:r0 + SUB, :], in_=o_t[:])

            def pf_trunk(c, hTv):
                """fT[m] = relu(Wpf.T @ concat(pvf_emb, ppf) + bpf)."""
                out = []
                for m in range(KH):
                    ps = pm1.tile([128, CHUNK], f32, tag="pm1", name=f"psf{c}_{m}")
                    for kk in range(KH):
                        nc.tensor.matmul(
                            out=ps[:],
                            lhsT=wpf_a[:, kk * H + m * 128: kk * H + (m + 1) * 128],
                            rhs=hTv[kk][:],
                            start=(kk == 0), stop=False)
                    nc.tensor.matmul(
                        out=ps[:],
                        lhsT=wpf_b[:, m * 128:(m + 1) * 128],
                        rhs=ppf_sb[:, c * CHUNK:(c + 1) * CHUNK],
                        start=False, stop=True)
                    f_t = hpool.tile([128, CHUNK], f32r, tag="hT", name=f"f{c}_{m}")
                    nc.scalar.activation(out=f_t[:], in_=ps[:], func=AF.Relu,
                                         bias=bpf_sb[:, m:m + 1])
                    out.append(f_t)
                return out

            def pred_head(c, fT):
                """p = pf_emb @ W_pred + so2p[idx] * exp(factor) -> DRAM."""
                for i in range(NSUB):
                    psp = pmp.tile([128, PRED], f32, tag="pmp", name=f"psp{c}_{i}")
                    for kk in range(KH):
                        nc.tensor.matmul(out=psp[:],
                                         lhsT=fT[kk][:, i * SUB:(i + 1) * SUB],
                                         rhs=wpred_sb[:, kk * PRED:(kk + 1) * PRED],
                                         start=(kk == 0), stop=(kk == KH - 1))
                    g_t = spool.tile([128, PRED], f32, tag="g", name=f"g{c}_{i}")
                    t = c * NSUB + i
                    nc.gpsimd.indirect_dma_start(
                        out=g_t[:], out_offset=None, in_=so2p[:],
                        in_offset=bass.IndirectOffsetOnAxis(
                            ap=idx_sb[:, t:t + 1], axis=0))
                    nc.vector.tensor_scalar(out=g_t[:], in0=g_t[:],
                                            scalar1=exp_sb[:, 0:1],
                                            scalar2=None, op0=ALU.mult)
                    p_t = spool.tile([128, PRED], f32, tag="p", name=f"p{c}_{i}")
                    nc.vector.tensor_tensor(out=p_t[:], in0=psp[:], in1=g_t[:],
                                            op=ALU.add)
                    r0 = c * CHUNK + i * SUB
                    nc.sync.dma_start(out=p_out[r0:r0 + SUB, :], in_=p_t[:])

            for c in range(NCHUNK):
                hTs = mm1("s", c)
                hTo = mm1("o", c)
                # prefetch next chunk's activations while this chunk computes
                if c + 1 < NCHUNK:
                    for path in ("s", "o", "v"):
                        load_x(path, srcs[path], c + 1)
                mm2("s", c, hTs)
                hTv = mm1("v", c)
                mm2("o", c, hTo)
                fT = pf_trunk(c, hTv)
                pred_head(c, fT)

    nc.compile()
    return nc


def _get_nc():
    global _BUILT
    if _BUILT is None:
        _BUILT = _build()
    return _BUILT


def _round_f32r(a):
    """Round fp32 to fp32r in place-compatible form: RNE to 11 mantissa bits
    (matches the on-device SWDGE fp32->fp32r cast bit-exactly up to ties)."""
    b = a.view(np.uint32)
    r = b + np.uint32(0x7FF) + ((b >> np.uint32(12)) & np.uint32(1))
    r &= np.uint32(0xFFFFF000)
    return r.view(np.float32)


def _block_x(x_shard):
    """[BC, F] -> [128, NCHUNK*KF*CHUNK]: (p, c, kk, b) = x[c*CHUNK+b, kk*128+p]."""
    xt = np.ascontiguousarray(x_shard.T)                  # [F, BC]
    xt = xt.reshape(KF, 128, NCHUNK, CHUNK).transpose(1, 2, 0, 3)
    return _round_f32r(np.ascontiguousarray(xt.reshape(128, NCHUNK * KF * CHUNK)))


def _block_w(w):
    """[K, N] -> [128, (K//128)*N]: (p, kk, n) = w[kk*128+p, n]."""
    K, N = w.shape
    wt = w.reshape(K // 128, 128, N).transpose(1, 0, 2)
    return _round_f32r(np.ascontiguousarray(wt.reshape(128, (K // 128) * N)))


def kernel(inp_sf, inp_of, inp_ppf, inp_pvf, gt_s, gt_o,
           W_obj1, b_obj1, W_obj2, W_pvf, b_pvf, W_pf, b_pf, W_pred,
           so2p, so2p_factor):
    global LAST_RESULT
    from concourse.bass_utils import run_bass_kernel_spmd

    f4 = np.float32
    inp_sf = np.asarray(inp_sf, f4)
    inp_of = np.asarray(inp_of, f4)
    inp_ppf = np.asarray(inp_ppf, f4)
    inp_pvf = np.asarray(inp_pvf, f4)
    gt_s = np.asarray(gt_s).astype(np.int32)
    gt_o = np.asarray(gt_o).astype(np.int32)
    so2p = np.asarray(so2p, f4)

    w2_pad = np.zeros((H, OBJP), f4)
    w2_pad[:, :OBJ] = np.asarray(W_obj2, f4)

    shared = {
        "w1": _block_w(np.asarray(W_obj1, f4)),
        "b1": np.ascontiguousarray(np.asarray(b_obj1, f4).reshape(KH, 128).T),
        "w2": _block_w(w2_pad),
        "wv": _block_w(np.asarray(W_pvf, f4)),
        "bv": np.ascontiguousarray(np.asarray(b_pvf, f4).reshape(KH, 128).T),
        "wpf": _block_w(np.asarray(W_pf, f4)[0:H]),
        "wpfb": _round_f32r(np.ascontiguousarray(np.asarray(W_pf, f4)[H:CAT])),
        "bpf": np.ascontiguousarray(np.asarray(b_pf, f4).reshape(KH, 128).T),
        "wpred": _block_w(np.asarray(W_pred, f4)),
        "so2p": np.ascontiguousarray(so2p),
        "factor": np.full((128, 1), np.asarray(so2p_factor, f4).reshape(-1)[0], f4),
    }

    in_maps = []
    for i in range(NCORES):
        sl = slice(i * BC, (i + 1) * BC)
        in_maps.append({
            **shared,
            "sfb": _block_x(inp_sf[sl]),
            "ofb": _block_x(inp_of[sl]),
            "pvfb": _block_x(inp_pvf[sl]),
            "ppfT": _round_f32r(np.ascontiguousarray(inp_ppf[sl].T)),
            "gts": np.ascontiguousarray(gt_s[sl].reshape(NT, SUB).T),
            "gto": np.ascontiguousarray(gt_o[sl].reshape(NT, SUB).T),
        })

    nc = _get_nc()
    res = run_bass_kernel_spmd(nc, in_maps, core_ids=list(range(NCORES)))
    LAST_RESULT = res

    s_out = np.concatenate([res.results[i]["s_score"] for i in range(NCORES)], axis=0)
    o_out = np.concatenate([res.results[i]["o_score"] for i in range(NCORES)], axis=0)
    p_res = np.concatenate([res.results[i]["p_out"] for i in range(NCORES)], axis=0)
    return (s_out, o_out, p_res)


# revision 13
# speedup vs baseline: 1.0761x; 1.0761x over previous
"""Trainium2 Bass kernel for nn_CascadeClassifier (embedding_lookup).

Computes, for a pair batch B=16384:
  s_score = relu(inp_sf @ W_obj1 + b_obj1) @ W_obj2          [B, 1001]
  o_score = relu(inp_of @ W_obj1 + b_obj1) @ W_obj2          [B, 1001]
  pvf_emb = relu(inp_pvf @ W_pvf + b_pvf)                    [B, 512]
  pf_emb  = relu(concat(pvf_emb, inp_ppf) @ W_pf + b_pf)     [B, 512]
  p_out   = pf_emb @ W_pred + so2p[gt_s*1001+gt_o] * exp(f)  [B, 132]

Sharding: data-parallel over B across 8 NeuronCores (2048 rows each);
MLP weights and the 529 MB so2p table replicated per core.  Activations
and weights are pre-blocked host-side into the exact SBUF tile layout
(feature-on-partition, k-tiles contiguous) so every SWDGE load is a
single max-width descriptor stream and no on-device transposes exist;
hidden activations are produced feature-on-partition (hT layout), which
makes layer biases per-partition ACT bias operands and feeds the next
matmul's stationary side directly.  Matmuls run in fp32r (full PE rate
at moving dim >= 256, ~2e-4 rel err vs 2.5e-3 for bf16).  The so2p
lookup is an on-device indirect-DMA row gather with indices computed on
VectorE.
"""

import numpy as np

# ---- problem constants (hardcoded per contract) ----
B = 16384
NCORES = 8
BC = B // NCORES          # 2048 rows per core
F = 1024                  # OBJ_FEAT / PVF_DIM
H = 512                   # HIDDEN
OBJ = 1001
OBJP = OBJ + 1            # even-padded: fp32r matmul dst needs even free dim
PRED = 132
PPF = 64
CAT = H + PPF             # 576
CHUNK = 512               # batch tile in the matmul free dim
NCHUNK = BC // CHUNK      # 4
SUB = 128                 # batch sub-tile (output partition dim)
NSUB = CHUNK // SUB       # 4
NT = BC // SUB            # 16 index tiles of 128
KF = F // 128             # 8 k-tiles for the F-dim matmuls
KH = H // 128             # 4 k-tiles for the H-dim matmuls
XW = KF * CHUNK           # per-chunk x-tile free width (4096)

_BUILT = None
LAST_RESULT = None


def _build():
    import concourse.bass as bass
    import concourse.tile as tile
    from concourse import bacc, mybir

    f32 = mybir.dt.float32
    f32r = mybir.dt.float32r
    i32 = mybir.dt.int32
    AF = mybir.ActivationFunctionType
    ALU = mybir.AluOpType

    nc = bacc.Bacc("TRN2", target_bir_lowering=False, debug=False,
                   num_devices=NCORES)

    # ---- per-core device IO (host pre-blocked layouts) ----
    sfb = nc.dram_tensor("sfb", [128, NCHUNK * XW], f32r, kind="ExternalInput")
    ofb = nc.dram_tensor("ofb", [128, NCHUNK * XW], f32r, kind="ExternalInput")
    pvfb = nc.dram_tensor("pvfb", [128, NCHUNK * XW], f32r, kind="ExternalInput")
    ppfT = nc.dram_tensor("ppfT", [PPF, BC], f32r, kind="ExternalInput")
    gts = nc.dram_tensor("gts", [SUB, NT], i32, kind="ExternalInput")
    gto = nc.dram_tensor("gto", [SUB, NT], i32, kind="ExternalInput")
    w1 = nc.dram_tensor("w1", [128, KF * H], f32r, kind="ExternalInput")
    b1 = nc.dram_tensor("b1", [128, KH], f32, kind="ExternalInput")
    w2 = nc.dram_tensor("w2", [128, KH * OBJP], f32r, kind="ExternalInput")
    wv = nc.dram_tensor("wv", [128, KF * H], f32r, kind="ExternalInput")
    bv = nc.dram_tensor("bv", [128, KH], f32, kind="ExternalInput")
    wpf = nc.dram_tensor("wpf", [128, KH * H], f32r, kind="ExternalInput")
    wpfb = nc.dram_tensor("wpfb", [PPF, H], f32r, kind="ExternalInput")
    bpf = nc.dram_tensor("bpf", [128, KH], f32, kind="ExternalInput")
    wpred = nc.dram_tensor("wpred", [128, KH * PRED], f32r, kind="ExternalInput")
    so2p = nc.dram_tensor("so2p", [OBJ * OBJ, PRED], f32, kind="ExternalInput")
    factor = nc.dram_tensor("factor", [128, 1], f32, kind="ExternalInput")

    s_score = nc.dram_tensor("s_score", [BC, OBJ], f32, kind="ExternalOutput")
    o_score = nc.dram_tensor("o_score", [BC, OBJ], f32, kind="ExternalOutput")
    p_out = nc.dram_tensor("p_out", [BC, PRED], f32, kind="ExternalOutput")

    with tile.TileContext(nc) as tc:
        with tc.tile_pool(name="wpool", bufs=1) as wpool, \
             tc.tile_pool(name="xpool", bufs=4) as xpool, \
             tc.tile_pool(name="hpool", bufs=18) as hpool, \
             tc.tile_pool(name="opool", bufs=4) as opool, \
             tc.tile_pool(name="spool", bufs=4) as spool, \
             tc.tile_pool(name="pm1", bufs=2, space="PSUM") as pm1, \
             tc.tile_pool(name="pm2", bufs=2, space="PSUM") as pm2, \
             tc.tile_pool(name="pmp", bufs=2, space="PSUM") as pmp:

            # ---- persistent weights ----
            # All fp32r operands are pre-rounded host-side (_round_f32r), so
            # these are plain cast-free DMAs.  Load order = SWDGE queue
            # order; sequenced so the first chunk's matmul dependencies
            # (w1, x_s0, x_o0) land earliest.
            # w1 + x_s0 gate the first matmuls: interleave their halves so
            # the k0-3 matmuls start while the second halves are in flight
            w1_sb = wpool.tile([128, KF * H], f32r)
            hw = KF * H // 2
            nc.gpsimd.dma_start(out=w1_sb[:, 0:hw], in_=w1[:, 0:hw])

            x_sb = {}
            x_eng = {"s": nc.gpsimd, "o": nc.gpsimd, "v": nc.gpsimd}

            def load_x(path, src, c):
                x_t = xpool.tile([128, XW], f32r, tag="x", name=f"x_{path}{c}")
                x_eng[path].dma_start(out=x_t[:], in_=src[:, c * XW:(c + 1) * XW])
                x_sb[(path, c)] = x_t

            x_t0 = xpool.tile([128, XW], f32r, tag="x", name="x_s0")
            nc.gpsimd.dma_start(out=x_t0[:, 0:XW // 2], in_=sfb[:, 0:XW // 2])
            nc.gpsimd.dma_start(out=w1_sb[:, hw:], in_=w1[:, hw:])
            nc.gpsimd.dma_start(out=x_t0[:, XW // 2:XW], in_=sfb[:, XW // 2:XW])
            x_sb[("s", 0)] = x_t0
            load_x("o", ofb, 0)

            w2_sb = wpool.tile([128, KH * OBJP], f32r)
            nc.gpsimd.dma_start(out=w2_sb[:], in_=w2[:])

            load_x("v", pvfb, 0)

            wv_sb = wpool.tile([128, KF * H], f32r)
            nc.gpsimd.dma_start(out=wv_sb[:], in_=wv[:])
            wpf_a = wpool.tile([128, KH * H], f32r)
            nc.gpsimd.dma_start(out=wpf_a[:], in_=wpf[:])
            wpf_b = wpool.tile([PPF, H], f32r)
            nc.gpsimd.dma_start(out=wpf_b[:], in_=wpfb[:])
            wpred_sb = wpool.tile([128, KH * PRED], f32r)
            nc.gpsimd.dma_start(out=wpred_sb[:], in_=wpred[:])
            ppf_sb = wpool.tile([PPF, BC], f32r)
            nc.gpsimd.dma_start(out=ppf_sb[:], in_=ppfT[:])

            # ---- small constants (HWDGE, parallel to the SWDGE queue) ----
            b1_sb = wpool.tile([128, KH], f32)
            nc.sync.dma_start(out=b1_sb[:], in_=b1[:])
            bv_sb = wpool.tile([128, KH], f32)
            nc.sync.dma_start(out=bv_sb[:], in_=bv[:])
            bpf_sb = wpool.tile([128, KH], f32)
            nc.sync.dma_start(out=bpf_sb[:], in_=bpf[:])
            fac_sb = wpool.tile([128, 1], f32)
            nc.sync.dma_start(out=fac_sb[:], in_=factor[:])
            exp_sb = wpool.tile([128, 1], f32)
            nc.scalar.activation(out=exp_sb[:], in_=fac_sb[:], func=AF.Exp)

            # ---- gather indices: idx = gt_s * 1001 + gt_o (int32, exact) ----
            gts_sb = wpool.tile([SUB, NT], i32)
            nc.sync.dma_start(out=gts_sb[:], in_=gts[:])
            gto_sb = wpool.tile([SUB, NT], i32)
            nc.sync.dma_start(out=gto_sb[:], in_=gto[:])
            idx_sb = wpool.tile([SUB, NT], i32)
            nc.vector.tensor_scalar(out=idx_sb[:], in0=gts_sb[:], scalar1=OBJ,
                                    scalar2=None, op0=ALU.mult)
            nc.vector.tensor_tensor(out=idx_sb[:], in0=idx_sb[:], in1=gto_sb[:],
                                    op=ALU.add)

            srcs = {"s": sfb, "o": ofb, "v": pvfb}
            w1s = {"s": w1_sb, "o": w1_sb, "v": wv_sb}
            b1s = {"s": b1_sb, "o": b1_sb, "v": bv_sb}

            def mm1(path, c):
                """layer 1: hT[m] = relu(W.T @ xT + b), 4 tiles [128h, CHUNK]."""
                x_t = x_sb.pop((path, c))
                out = []
                for m in range(KH):
                    ps = pm1.tile([128, CHUNK], f32, tag="pm1",
                                  name=f"ps1_{path}{c}_{m}")
                    for kk in range(KF):
                        nc.tensor.matmul(
                            out=ps[:],
                            lhsT=w1s[path][:, kk * H + m * 128: kk * H + (m + 1) * 128],
                            rhs=x_t[:, kk * CHUNK:(kk + 1) * CHUNK],
                            start=(kk == 0), stop=(kk == KF - 1))
                    h_t = hpool.tile([128, CHUNK], f32r, tag="hT",
                                     name=f"h_{path}{c}_{m}")
                    nc.scalar.activation(out=h_t[:], in_=ps[:], func=AF.Relu,
                                         bias=b1s[path][:, m:m + 1])
                    out.append(h_t)
                return out

            def mm2(path, c, hT):
                """layer 2 for s/o: [128b, OBJ] score tiles -> DRAM."""
                out_dram = s_score if path == "s" else o_score
                for i in range(NSUB):
                    ps2 = pm2.tile([128, OBJP], f32, tag="pm2",
                                   name=f"ps2_{path}{c}_{i}")
                    for kk in range(KH):
                        lhsT = hT[kk][:, i * SUB:(i + 1) * SUB]
                        nc.tensor.matmul(out=ps2[:, 0:512],
                                         lhsT=lhsT,
                                         rhs=w2_sb[:, kk * OBJP: kk * OBJP + 512],
                                         start=(kk == 0), stop=(kk == KH - 1))
                        nc.tensor.matmul(out=ps2[:, 512:OBJP],
                                         lhsT=lhsT,
                                         rhs=w2_sb[:, kk * OBJP + 512:(kk + 1) * OBJP],
                                         start=(kk == 0), stop=(kk == KH - 1))
                    o_t = opool.tile([128, OBJ], f32, tag="out",
                                     name=f"o_{path}{c}_{i}")
                    nc.vector.tensor_copy(out=o_t[:], in_=ps2[:, 0:OBJ])
                    r0 = c * CHUNK + i * SUB
                    nc.sync.dma_start(out=out_dram[r0:r0 + SUB, :], in_=o_t[:])

            def pf_trunk(c, hTv):
                """fT[m] = relu(Wpf.T @ concat(pvf_emb, ppf) + bpf)."""
                out = []
                for m in range(KH):
                    ps = pm1.tile([128, CHUNK], f32, tag="pm1", name=f"psf{c}_{m}")
                    for kk in range(KH):
                        nc.tensor.matmul(
                            out=ps[:],
                            lhsT=wpf_a[:, kk * H + m * 128: kk * H + (m + 1) * 128],
                            rhs=hTv[kk][:],
                            start=(kk == 0), stop=False)
                    nc.tensor.matmul(
                        out=ps[:],
                        lhsT=wpf_b[:, m * 128:(m + 1) * 128],
                        rhs=ppf_sb[:, c * CHUNK:(c + 1) * CHUNK],
                        start=False, stop=True)
                    f_t = hpool.tile([128, CHUNK], f32r, tag="hT", name=f"f{c}_{m}")
                    nc.scalar.activation(out=f_t[:], in_=ps[:], func=AF.Relu,
                                         bias=bpf_sb[:, m:m + 1])
                    out.append(f_t)
                return out

            def pred_head(c, fT):
                """p = pf_emb @ W_pred + so2p[idx] * exp(factor) -> DRAM."""
                for i in range(NSUB):
                    psp = pmp.tile([128, PRED], f32, tag="pmp", name=f"psp{c}_{i}")
                    for kk in range(KH):
                        nc.tensor.matmul(out=psp[:],
                                         lhsT=fT[kk][:, i * SUB:(i + 1) * SUB],
                                         rhs=wpred_sb[:, kk * PRED:(kk + 1) * PRED],
                                         start=(kk == 0), stop=(kk == KH - 1))
                    g_t = spool.tile([128, PRED], f32, tag="g", name=f"g{c}_{i}")
                    t = c * NSUB + i
                    nc.gpsimd.indirect_dma_start(
                        out=g_t[:], out_offset=None, in_=so2p[:],
                        in_offset=bass.IndirectOffsetOnAxis(
                            ap=idx_sb[:, t:t + 1], axis=0))
                    nc.vector.tensor_scalar(out=g_t[:], in0=g_t[:],
                                            scalar1=exp_sb[:, 0:1],
                                            scalar2=None, op0=ALU.mult)
                    p_t = spool.tile([128, PRED], f32, tag="p", name=f"p{c}_{i}")
                    nc.vector.tensor_tensor(out=p_t[:], in0=psp[:], in1=g_t[:],
                                            op=ALU.add)
                    r0 = c * CHUNK + i * SUB
                    nc.sync.dma_start(out=p_out[r0:r0 + SUB, :], in_=p_t[:])

            for c in range(NCHUNK):
                hTs = mm1("s", c)
                hTo = mm1("o", c)
                # prefetch next chunk's activations while this chunk computes
                if c + 1 < NCHUNK:
                    for path in ("s", "o", "v"):
                        load_x(path, srcs[path], c + 1)
                mm2("s", c, hTs)
                hTv = mm1("v", c)
                mm2("o", c, hTo)
                fT = pf_trunk(c, hTv)
                pred_head(c, fT)

    nc.compile()
    return nc


def _get_nc():
    global _BUILT
    if _BUILT is None:
        _BUILT = _build()
    return _BUILT


def _round_f32r(a):
    """Round fp32 to fp32r in place-compatible form: RNE to 11 mantissa bits
    (matches the on-device SWDGE fp32->fp32r cast bit-exactly up to ties)."""
    b = a.view(np.uint32)
    r = b + np.uint32(0x7FF) + ((b >> np.uint32(12)) & np.uint32(1))
    r &= np.uint32(0xFFFFF000)
    return r.view(np.float32)


def _block_x(x_shard):
    """[BC, F] -> [128, NCHUNK*KF*CHUNK]: (p, c, kk, b) = x[c*CHUNK+b, kk*128+p]."""
    xt = np.ascontiguousarray(x_shard.T)                  # [F, BC]
    xt = xt.reshape(KF, 128, NCHUNK, CHUNK).transpose(1, 2, 0, 3)
    return _round_f32r(np.ascontiguousarray(xt.reshape(128, NCHUNK * KF * CHUNK)))


def _block_w(w):
    """[K, N] -> [128, (K//128)*N]: (p, kk, n) = w[kk*128+p, n]."""
    K, N = w.shape
    wt = w.reshape(K // 128, 128, N).transpose(1, 0, 2)
    return _round_f32r(np.ascontiguousarray(wt.reshape(128, (K // 128) * N)))


def kernel(inp_sf, inp_of, inp_ppf, inp_pvf, gt_s, gt_o,
           W_obj1, b_obj1, W_obj2, W_pvf, b_pvf, W_pf, b_pf, W_pred,
           so2p, so2p_factor):
    global LAST_RESULT
    from concourse.bass_utils import run_bass_kernel_spmd

    f4 = np.float32
    inp_sf = np.asarray(inp_sf, f4)
    inp_of = np.asarray(inp_of, f4)
    inp_ppf = np.asarray(inp_ppf, f4)
    inp_pvf = np.asarray(inp_pvf, f4)
    gt_s = np.asarray(gt_s).astype(np.int32)
    gt_o = np.asarray(gt_o).astype(np.int32)
    so2p = np.asarray(so2p, f4)

    w2_pad = np.zeros((H, OBJP), f4)
    w2_pad[:, :OBJ] = np.asarray(W_obj2, f4)

    shared = {
        "w1": _block_w(np.asarray(W_obj1, f4)),
        "b1": np.ascontiguousarray(np.asarray(b_obj1, f4).reshape(KH, 128).T),
        "w2": _block_w(w2_pad),
        "wv": _block_w(np.asarray(W_pvf, f4)),
        "bv": np.ascontiguousarray(np.asarray(b_pvf, f4).reshape(KH, 128).T),
        "wpf": _block_w(np.asarray(W_pf, f4)[0:H]),
        "wpfb": _round_f32r(np.ascontiguousarray(np.asarray(W_pf, f4)[H:CAT])),
        "bpf": np.ascontiguousarray(np.asarray(b_pf, f4).reshape(KH, 128).T),
        "wpred": _block_w(np.asarray(W_pred, f4)),
        "so2p": np.ascontiguousarray(so2p),
        "factor": np.full((128, 1), np.asarray(so2p_factor, f4).reshape(-1)[0], f4),
    }

    in_maps = []
    for i in range(NCORES):
        sl = slice(i * BC, (i + 1) * BC)
        in_maps.append({
            **shared,
            "sfb": _block_x(inp_sf[sl]),
            "ofb": _block_x(inp_of[sl]),
            "pvfb": _block_x(inp_pvf[sl]),
            "ppfT": _round_f32r(np.ascontiguousarray(inp_ppf[sl].T)),
            "gts": np.ascontiguousarray(gt_s[sl].reshape(NT, SUB).T),
            "gto": np.ascontiguousarray(gt_o[sl].reshape(NT, SUB).T),
        })

    nc = _get_nc()
    res = run_bass_kernel_spmd(nc, in_maps, core_ids=list(range(NCORES)))
    LAST_RESULT = res

    s_out = np.concatenate([res.results[i]["s_score"] for i in range(NCORES)], axis=0)
    o_out = np.concatenate([res.results[i]["o_score"] for i in range(NCORES)], axis=0)
    p_res = np.concatenate([res.results[i]["p_out"] for i in range(NCORES)], axis=0)
    return (s_out, o_out, p_res)
